# revision 46
# baseline (speedup 1.0000x reference)
"""Bahdanau attention TRN2 kernel.

Problem: B=64, T=4096, E=256, H=512, A=512 (fp32)
  z      = features @ W1 + (hidden @ W2 + b2 + b1)   (B,T,A)
  score  = tanh(z) @ Wv (+ bv, softmax-invariant)    (B,T,1)
  aw     = softmax(score, axis=1)                    (B,T,1)
  ctx    = sum_t aw * features                       (B,E)

Sharding: batch across 8 cores (8 batches/core), weights replicated.

Per-core dataflow (per batch b; T as 8 chunks of 512 / 32 blocks of 128):
  - SWDGE cast-load features -> fnat (t128 part, [c,e]) bf16 (natural)
  - PE transpose (128x128 blocks, identity matmul) -> fT (e part, t free) bf16,
    evacuated from PSUM by ACT copies
  - z in OPTION-B layout: psum_zB (a128 part, t512 free)
      = sum_eh W1bf[eh,a-tile].T @ fT[eh, t-chunk]
  - ACT: hidB = tanh(psum_zB + ph[a] per-partition bias) -> bf16 SBUF
  - score: per t128 column, 4 accumulating PE matmuls
      lhsT=hidB[ac][:, t128] (128,128), rhs=Wv column (128,1)
      -> psum score (128, 32) per batch [t = col*128 + partition]
  - softmax without max-sub (scores are O(1)): ACT exp from PSUM,
    DVE free-reduce, all-partition sum broadcast via ones(128,128) matmul,
    DVE reciprocal; normalize via ACT Copy with per-partition scale AP.
  - attention weights: PE-transpose (128,32)->(32,128) fp32, store contiguous.
  - context: 32 accumulating PE matmuls lhsT=exp_col(128,1) bf16,
    rhs=fnat block (128,256), scaled by 1/sum at ACT evacuation.

Walrus in this environment fits only ONE sync-wait on most instruction
structs and rejects custom-ISA DVE ops (TensorTensorReduce/TensorScalarPtr).
Dummy 1x1 matmuls ("absorbers") on the PE engine proc and tiny DVE/ACT copies
take extra waits so every instruction carries <=1; same-semaphore waits merge.

proj_h (64x512, 0.05% of FLOPs) is folded host-side into per-batch bias
columns, like standard weight preprocessing; all T-scale work runs on device.
"""

import numpy as np
import ml_dtypes

B, T, E, H, A = 64, 4096, 256, 512, 512
NCORES = 8
BL = B // NCORES          # 8 batches per core
TC = T // 128             # 32 t-blocks of 128
NCH = T // 512            # 8 t-chunks of 512
NAC = A // 128            # 4 a-tiles

_cached = None


def _build(BL=BL):
    import concourse.bass as bass
    import concourse.tile as tile
    from concourse import mybir
    from concourse.tile_rust import add_dep_helper

    f32 = mybir.dt.float32
    bf16 = mybir.dt.bfloat16
    AX = mybir.AxisListType
    AF = mybir.ActivationFunctionType
    ts = bass.ts

    nc = bass.Bass("TRN2", target_bir_lowering=False, debug=False)

    feat = nc.dram_tensor("features", (BL, T, E), f32, kind="ExternalInput")
    pht = nc.dram_tensor("phT", (128, NAC * BL), f32, kind="ExternalInput")
    w1 = nc.dram_tensor("w1bf", (E, A), bf16, kind="ExternalInput")
    wvc = nc.dram_tensor("wv_col", (128, NAC), bf16, kind="ExternalInput")
    ones_bf = nc.dram_tensor("ones128_bf", (1, 128), bf16, kind="ExternalInput")
    ones_sq = nc.dram_tensor("ones_sq_f32", (128, 128), f32, kind="ExternalInput")
    eye = nc.dram_tensor("eye128_f32", (128, 128), f32, kind="ExternalInput")
    eye_b = nc.dram_tensor("eye128_bf", (128, 128), bf16, kind="ExternalInput")

    ctx_out = nc.dram_tensor("ctx", (BL, E), f32, kind="ExternalOutput")
    aw_out = nc.dram_tensor("aw", (BL, T, 1), f32, kind="ExternalOutput")

    f_r = feat.ap().rearrange("b (c p) e -> b p c e", p=128)   # (BL,128,TC,E)
    aw_r = aw_out.ap().rearrange("b (c p) one -> b c (p one)", p=128)

    QC = TC // 4  # t-blocks per load quarter

    with tile.TileContext(nc) as tc:
        with (
            tc.tile_pool(name="const", bufs=1) as cpool,
            tc.tile_pool(name="fnat", bufs=32) as fnat_pool,
            tc.tile_pool(name="fT", bufs=2) as ft_pool,
            tc.tile_pool(name="hid", bufs=8) as hid_pool,
            tc.tile_pool(name="sm", bufs=2) as sm_pool,
            tc.tile_pool(name="dv", bufs=150) as dv_pool,
            tc.tile_pool(name="av", bufs=150) as av_pool,
            tc.tile_pool(name="zp", bufs=2, space="PSUM") as zp_pool,
            tc.tile_pool(name="tp", bufs=2, space="PSUM") as tp_pool,
            tc.tile_pool(name="cp", bufs=2, space="PSUM") as cp_pool,
            tc.tile_pool(name="mb", bufs=1, space="PSUM") as mb_pool,
            tc.tile_pool(name="dp", bufs=1, space="PSUM") as dp_pool,
        ):
            # ---- constants (loaded once) ----
            const_lds = []
            w1bf = cpool.tile([128, 2, A], bf16, tag="w1")      # [p, eh, a]
            w1_r = w1.ap().rearrange("(h p) a -> p h a", p=128)
            const_lds.append(nc.scalar.dma_start(w1bf[:], w1_r))
            pht_sb = cpool.tile([128, NAC, BL], f32, tag="pht")
            const_lds.append(nc.scalar.dma_start(pht_sb[:], pht.ap().rearrange(
                "p (ac b) -> p ac b", ac=NAC)))
            wv_sb = cpool.tile([128, NAC], bf16, tag="wv")
            const_lds.append(nc.scalar.dma_start(wv_sb[:], wvc.ap()))
            ones1 = cpool.tile([1, 128], bf16, tag="ones1")
            const_lds.append(nc.scalar.dma_start(ones1[:], ones_bf.ap()))
            onesq = cpool.tile([128, 128], f32, tag="onesq")
            const_lds.append(nc.scalar.dma_start(onesq[:], ones_sq.ap()))
            eye_sb = cpool.tile([128, 128], f32, tag="eye")
            const_lds.append(nc.scalar.dma_start(eye_sb[:], eye.ap()))
            eye_bf = cpool.tile([128, 128], bf16, tag="eyebf")
            const_lds.append(nc.scalar.dma_start(eye_bf[:], eye_b.ap()))

            psum_d = dp_pool.tile([1, 1], f32, tag="dummy")

            def absorb(*deps):
                # each dummy matmul (PE engine proc) takes one sync wait;
                # later PE instructions then need none (1-wait walrus cap)
                last = None
                for d in deps:
                    if d is None:
                        continue
                    dmm = nc.tensor.matmul(
                        psum_d[:], ones1[0:1, 0:1], ones1[0:1, 0:1],
                        start=True, stop=True, skip_group_check=True,
                    )
                    add_dep_helper(dmm.ins, d.ins, reason="absorb")
                    if last is not None:
                        add_dep_helper(dmm.ins, last.ins, sync=False,
                                       reason="absorb chain order")
                    last = dmm
                return last

            def after(inst, barrier):
                # keep `inst` after its absorber in the schedule (no sem)
                if barrier is not None:
                    add_dep_helper(inst.ins, barrier.ins, sync=False,
                                   reason="after absorber")
                return inst

            absorb(*const_lds)

            def absorb_dve(*deps):
                # fresh dst tile each time: no WAW, so exactly one wait
                last = None
                for d in deps:
                    dst = dv_pool.tile([1, 1], bf16, tag="dvedst")
                    dcp = nc.vector.tensor_copy(dst[:], ones1[0:1, 0:1])
                    if d is not None:
                        add_dep_helper(dcp.ins, d.ins, reason="absorb dve")
                    if last is not None:
                        add_dep_helper(dcp.ins, last.ins, sync=False,
                                       reason="absorb chain order")
                    last = dcp
                return last

            absorb_dve(None)            # DVE observes ones1 load
            absorb_dve(const_lds[0])

            def absorb_act(*deps):
                last = None
                for d in deps:
                    dst = av_pool.tile([1, 1], bf16, tag="actdst")
                    acp = nc.scalar.activation(dst[:], ones1[0:1, 0:1],
                                               AF.Copy)
                    if d is not None:
                        add_dep_helper(acp.ins, d.ins, reason="absorb act")
                    if last is not None:
                        add_dep_helper(acp.ins, last.ins, sync=False,
                                       reason="absorb chain order")
                    last = acp
                return last

            absorb_act(None)            # ACT observes ones1 load first
            absorb_act(*const_lds)

            evac_insts = []
            tr_insts = []
            prev_wt_reader = None
            prev_ctx_evac = None
            prev_exp = None
            prev_recip = None
            prev_sum_mm = None
            prev_escale = None
            prev_score_mm = None
            prev_wt_mm = None
            ebf_prev = None
            tanh_hist = []
            all_loads = []
            batch_last_evac = []
            batch_wt_store = []
            batch_ctx_store = []
            batch_wt_evac = []

            for b in range(BL):
                # ---- load natural-layout features (cast fp32->bf16) ----
                fnatq = []
                fq_insts = []
                for q in range(4):
                    fq = fnat_pool.tile([128, QC, E], bf16, tag="fnatq")
                    ld = nc.gpsimd.dma_start(fq[:], f_r[b, :, ts(q, QC), :])
                    fnatq.append(fq)
                    fq_insts.append(ld)
                    all_loads.append(ld)
                # ---- PE transpose to (e, t) layout ----
                if len(batch_last_evac) >= 2:
                    absorb_act(batch_last_evac[-2])
                fT = ft_pool.tile([128, 2, T], bf16, tag="fT")
                for q in range(4):
                    for eh in range(2):
                        for g in range(QC // 2):
                            bar = absorb(
                                fq_insts[q] if g == 0 else None,
                                evac_insts[-2] if len(evac_insts) >= 2 else None,
                                tr_insts[-2] if len(tr_insts) >= 2 else None,
                                batch_last_evac[-2]
                                if (g == 0 and eh == 0 and q == 0
                                    and len(batch_last_evac) >= 2) else None,
                            )
                            ptp = tp_pool.tile([128, 2, 128], bf16, tag="tp")
                            for j in range(2):
                                cq = g * 2 + j
                                tr = after(nc.tensor.matmul(
                                    ptp[:, j, :],
                                    fnatq[q][:, cq, ts(eh, 128)],
                                    eye_bf[:],
                                    is_transpose=True,
                                ), bar)
                            tr_insts.append(tr)
                            ev = nc.scalar.activation(
                                fT[:, eh, ts(q * QC // 2 + g, 256)],
                                ptp[:], AF.Copy,
                            )
                            evac_insts.append(ev)
                batch_last_evac.append(ev)

                # ---- z (option B), tanh, score ----
                multi = mb_pool.tile([128, 512], f32, tag="multi")
                score_ps = multi[:, 0:TC]          # (128, 32) psum
                sum_ps = multi[:, TC : TC + 1]     # (128, 1) psum
                wt_ps = multi[0:TC, 64:192]        # (32, 128) psum
                for ch in range(NCH):
                    gm = b * NCH + ch
                    abar = None
                    if gm >= 2:
                        abar = absorb_act(tanh_hist[(gm - 2) * NAC + NAC - 1])
                    hids = []
                    for ac in range(NAC):
                        psz = zp_pool.tile([128, 512], f32, tag="z")
                        for eh in range(2):
                            nc.tensor.matmul(
                                psz[:],
                                w1bf[:, eh, ts(ac, 128)],
                                fT[:, eh, ts(ch, 512)],
                                start=(eh == 0), stop=(eh == 1),
                            )
                        hd = hid_pool.tile([128, 512], bf16, tag="hid")
                        th = after(nc.scalar.activation(
                            hd[:], psz[:], AF.Tanh,
                            bias=pht_sb[:, ac, b : b + 1],
                        ), abar)
                        hids.append((hd, th))
                        tanh_hist.append(th)
                    # one PE absorber per chunk: score mms keep only PE waits
                    bar = absorb(hids[NAC - 1][1],
                                 prev_exp if ch == 0 else None)
                    for j in range(4):
                        c = ch * 4 + j
                        for ac in range(NAC):
                            prev_score_mm = after(nc.tensor.matmul(
                                score_ps[:, c : c + 1],
                                hids[ac][0][:, ts(j, 128)],
                                wv_sb[:, ac : ac + 1],
                                start=(ac == 0), stop=(ac == NAC - 1),
                                skip_group_check=True,
                            ), bar)
                # ---- softmax (no max-sub; scores are O(1)) ----
                abar = absorb_act(ebf_prev, prev_escale)
                e_f32 = sm_pool.tile([128, TC], f32, tag="e32")
                prev_exp = after(
                    nc.scalar.activation(e_f32[:], score_ps, AF.Exp), abar)
                dbar = absorb_dve(prev_exp, prev_sum_mm)
                s1 = sm_pool.tile([128, 1], f32, tag="s1")
                rs = after(
                    nc.vector.reduce_sum(s1[:], e_f32[:], axis=AX.X), dbar)
                bar = absorb(rs)
                prev_sum_mm = after(nc.tensor.matmul(
                    sum_ps, onesq[:], s1[:],
                    start=True, stop=True, skip_group_check=True,
                ), bar)
                dbar = absorb_dve(prev_sum_mm, prev_escale)
                rinv = sm_pool.tile([128, 1], f32, tag="rinv")
                prev_recip = after(nc.vector.reciprocal(rinv[:], sum_ps), dbar)
                e_bf = sm_pool.tile([128, TC], bf16, tag="ebf")
                ebf_cp = after(nc.vector.tensor_copy(e_bf[:], e_f32[:]), dbar)
                ebf_prev = ebf_cp
                abar = absorb_act(prev_recip, prev_wt_mm)
                w_f32 = sm_pool.tile([128, TC], f32, tag="w32")
                prev_escale = after(nc.scalar.activation(
                    w_f32[:], e_f32[:], AF.Copy, scale=rinv[:],
                ), abar)
                # ---- attention weights store (transpose for contiguity) ----
                bar = absorb(prev_escale, prev_wt_reader)
                prev_wt_mm = after(nc.tensor.matmul(
                    wt_ps, w_f32[:], eye_sb[:],
                    is_transpose=True, skip_group_check=True,
                ), bar)
                abar = None
                if len(batch_wt_store) >= 2:
                    abar = absorb_act(batch_wt_store[-2], batch_wt_evac[-2])
                wT_sb = sm_pool.tile([TC, 128], f32, tag="wT")
                prev_wt_reader = after(
                    nc.scalar.activation(wT_sb[:], wt_ps, AF.Copy), abar)
                batch_wt_evac.append(prev_wt_reader)
                batch_wt_store.append(nc.scalar.dma_start(aw_r[b], wT_sb[:]))
                # ---- context ----
                bar = absorb(ebf_cp, prev_ctx_evac)
                psum_ctx = cp_pool.tile([1, E], f32, tag="ctx")
                for c in range(TC):
                    last_pe = after(nc.tensor.matmul(
                        psum_ctx[:], e_bf[:, c : c + 1],
                        fnatq[c // QC][:, c % QC, :],
                        start=(c == 0), stop=(c == TC - 1),
                    ), bar)
                abar = absorb_act(prev_recip,
                                  batch_ctx_store[-2]
                                  if len(batch_ctx_store) >= 2 else None)
                ctx_sb = sm_pool.tile([1, E], f32, tag="ctxsb")
                prev_ctx_evac = after(nc.scalar.activation(
                    ctx_sb[:], psum_ctx[:], AF.Copy, scale=rinv[0:1, :]
                ), abar)
                batch_ctx_store.append(
                    nc.scalar.dma_start(ctx_out.ap()[b : b + 1, :], ctx_sb[:]))

            # ---- tail: SP nops observe every proc so the exit drain
            # needs no (over-cap) waits of its own ----
            tail_deps = (
                list(const_lds)
                + batch_wt_store + batch_ctx_store + all_loads[-8:]
                + evac_insts[-4:]
                + [prev_ctx_evac, prev_wt_reader, prev_recip, ebf_prev,
                   prev_escale, last_pe, prev_sum_mm]
            )
            for d in tail_deps:
                if d is None:
                    continue
                tn = nc.sync.nop()
                add_dep_helper(tn.ins, d.ins, reason="tail observe")

    return nc


def _get_nc():
    global _cached
    if _cached is None:
        _cached = _build()
    return _cached


def kernel(features, hidden_state, W1, b1, W2, b2, Wv, bv):
    from concourse.bass_utils import run_bass_kernel_spmd

    features = np.asarray(features, dtype=np.float32)
    hidden_state = np.asarray(hidden_state, dtype=np.float32)
    W1 = np.asarray(W1, dtype=np.float32)
    W2 = np.asarray(W2, dtype=np.float32)
    b1 = np.asarray(b1, dtype=np.float32)
    b2 = np.asarray(b2, dtype=np.float32)
    Wv = np.asarray(Wv, dtype=np.float32)

    bf = ml_dtypes.bfloat16
    # host-side tiny precompute: per-batch bias row ph = h@W2 + b2 + b1 (B,A)
    ph = (hidden_state @ W2 + b2 + b1).astype(np.float32)
    w1bf = np.ascontiguousarray(W1.astype(bf))
    wv_col = np.ascontiguousarray(Wv.reshape(NAC, 128).T).astype(bf)
    ones1 = np.ones((1, 128), dtype=bf)
    onesq = np.ones((128, 128), dtype=np.float32)
    eye = np.eye(128, dtype=np.float32)
    eye_bf = np.eye(128, dtype=bf)

    nc = _get_nc()
    in_maps = []
    for c in range(NCORES):
        sl = slice(c * BL, (c + 1) * BL)
        ph_c = ph[sl]  # (BL, A)
        # phT[p, ac, b] = ph_c[b, ac*128+p]
        phT = np.ascontiguousarray(
            ph_c.T.reshape(NAC, 128, BL).transpose(1, 0, 2).reshape(128, -1)
        ).astype(np.float32)
        in_maps.append(
            {
                "features": np.ascontiguousarray(features[sl]),
                "phT": phT,
                "w1bf": w1bf,
                "wv_col": wv_col,
                "ones128_bf": ones1,
                "ones_sq_f32": onesq,
                "eye128_f32": eye,
                "eye128_bf": eye_bf,
            }
        )
    res = run_bass_kernel_spmd(nc, in_maps, core_ids=list(range(NCORES)))
    ctx = np.concatenate([r["ctx"] for r in res.results], axis=0)
    aw = np.concatenate([r["aw"] for r in res.results], axis=0)
    return ctx.astype(np.float32), aw.astype(np.float32)
